# revision 2
# baseline (speedup 1.0000x reference)
"""PSROIPool Trainium2 kernel v2: per-(tile,q) x-windows.

8-core SPMD data-parallel over ROIs (contiguous batch-sorted 256-ROI
chunks, <=2 images per core, 2-slot y-mask trick). Each core's 256 ROIs
are sorted by their last-bin right edge (we6) and split into 2 tiles of
128. For every (tile, q) a GLOBAL compile-time x-window [X0, X0+Wp) is
computed on the host (union over all cores, padded to a common width Wp)
that covers bin q of every ROI in that tile on every core. Stage 1 runs
one matmul per (tile, ph, q) with rhs = feat[ph, c=0..4, q, window]
(N = 5*Wp), so the masked x-reduction downstream only touches Wp
columns per bin instead of 56/64. mwr shrinks from [r, t*OD*GS*56] to
[r, t*GS*OD*Wp] (w-mask within the window, broadcast over c on host).
ScalarE casts PSUM->SBUF bf16; DVE does w-mask mul at 2x + bf16
add-tree + fp32 reduce. Falls back to full-width windows if the data
doesn't admit Wp <= 48.
"""

import numpy as np
import ml_dtypes

import concourse.bass as bass
import concourse.bacc as bacc
import concourse.mybir as mybir
import concourse.tile as tile
from concourse.bass_utils import run_bass_kernel_spmd
from contextlib import ExitStack

N_IMG = 8
OD = 5
GS = 7
C = OD * GS * GS  # 245
H = W = 64
R = 2048
SS = 1.0 / 16.0
N_CORES = 8
F32 = mybir.dt.float32
BF16 = mybir.dt.bfloat16
NPBF16 = ml_dtypes.bfloat16

_NC_CACHE: dict = {}


def _build_nc(rt, reps, Wp, X0, stages="full"):
    """X0: [rt][GS] compile-time window starts; Wp: common width."""
    assert rt == 2
    nc = bacc.Bacc()
    chx = C * W  # 15680
    NW = OD * Wp          # matmul N per (t, ph, q)
    SLOT = 256 if NW <= 256 else 512  # psum cols per q slot
    assert NW <= SLOT
    PHSEG = GS * NW       # stg cols per (t, ph): (q, c, x)
    NSEG = rt * GS * OD   # segments per ph op: (t, q, c)

    feat2 = nc.declare_dram_parameter("feat2", [128, chx], BF16, isOutput=False)
    mh = nc.declare_dram_parameter("mh", [128, rt * GS * 128], BF16, isOutput=False)
    mwr = nc.declare_dram_parameter("mwr", [128, 2 * rt * GS * OD * Wp], BF16, isOutput=False)
    outp = nc.declare_dram_parameter("out", [128, rt * GS * GS * OD], F32, isOutput=True)

    with tile.TileContext(nc) as tc:
        with ExitStack() as ctx:
            pool = ctx.enter_context(tc.tile_pool(name="sb", bufs=1 if reps == 1 else 2))
            stp = ctx.enter_context(tc.tile_pool(name="stg", bufs=2))
            prp = ctx.enter_context(tc.tile_pool(name="prd", bufs=2))
            trp = ctx.enter_context(tc.tile_pool(name="tr", bufs=2))
            psp = ctx.enter_context(
                tc.tile_pool(
                    name="ps", bufs=2 if SLOT <= 256 else 1,
                    space=bass.MemorySpace.PSUM,
                )
            )

            for _rep in range(reps):
                mhT = pool.tile([128, rt * GS * 128], BF16, tag="mh")
                nc.sync.dma_start(mhT[:], mh[:])
                mwT = pool.tile([128, 2 * rt * GS * OD * Wp], BF16, tag="mw")
                nc.sync.dma_start(mwT[:], mwr[:])
                featT = pool.tile([128, chx], BF16, tag="feat")
                PHW = OD * GS * W  # 2240 cols per ph
                for phc in range(GS):
                    nc.sync.dma_start(
                        featT[:, phc * PHW : (phc + 1) * PHW],
                        feat2[:, phc * PHW : (phc + 1) * PHW],
                    )
                outT = pool.tile([128, rt * GS * GS * OD], F32, tag="out")

                featv = featT[:].rearrange(
                    "p (g c q x) -> p g c q x", g=GS, c=OD, q=GS
                )
                outv = outT[:].rearrange(
                    "p (t h q c) -> p t h q c", t=rt, h=GS, q=GS
                )

                for php in ((0, 1), (2, 3), (4, 5), (6,)):
                    nj = len(php)
                    stg = stp.tile([128, nj * rt * PHSEG], BF16, tag="stg")
                    for j, ph in enumerate(php):
                        for t in range(rt):
                            k = t * GS + ph
                            lhs = mhT[:, k * 128 : (k + 1) * 128]
                            ps = psp.tile([128, GS * SLOT], F32, tag="ps")
                            psv = ps[:].rearrange("p (q s) -> p q s", s=SLOT)
                            for q in range(GS):
                                x0 = X0[t][q]
                                rhs = featv[:, ph, :, q, x0 : x0 + Wp]
                                nc.tensor.matmul(
                                    psv[:, q, 0:NW], lhs, rhs, start=True, stop=True
                                )
                            if stages in ("act", "full"):
                                nc.scalar.copy(
                                    stg[
                                        :,
                                        (j * rt + t) * PHSEG : (j * rt + t + 1) * PHSEG,
                                    ],
                                    psv[:, :, 0:NW],
                                )
                            elif stages == "pe":
                                nc.vector.tensor_add(
                                    stg[:, t : t + 1], psv[:, 0, 0:1], psv[:, 1, 1:2]
                                )
                    if stages != "full":
                        continue
                    prod = prp.tile([128, nj * rt * PHSEG], BF16, tag="prd")
                    nc.vector.tensor_mul(prod[:], stg[:], mwT[:, 0 : nj * rt * PHSEG])
                    pv = prod[:].rearrange("p (s x) -> p s x", x=Wp)
                    w = Wp
                    src = pv
                    lvl = 0
                    while w % 2 == 0 and w > 5:
                        h = w // 2
                        tl = trp.tile([128, nj * NSEG * h], BF16, tag=f"t{lvl}")
                        tlv = tl[:].rearrange("p (s x) -> p s x", x=h)
                        nc.vector.tensor_add(tlv, src[:, :, 0:h], src[:, :, h:w])
                        src = tlv
                        w = h
                        lvl += 1
                    nc.vector.reduce_sum(
                        outv[:, :, php[0] : php[0] + nj, :, :].rearrange(
                            "p t j q c -> p j t q c"
                        ),
                        src.rearrange(
                            "p (j t q c) x -> p j t q c x", j=nj, t=rt, q=GS
                        ),
                        axis=mybir.AxisListType.X,
                    )
                if stages == "full":
                    nc.sync.dma_start(outp[:], outT[:])

    nc.finalize()
    return nc


def _get_nc(rt, reps, Wp, X0, stages="full"):
    key = (rt, reps, Wp, tuple(map(tuple, X0)), stages)
    if key not in _NC_CACHE:
        _NC_CACHE[key] = _build_nc(rt, reps, Wp, X0, stages)
    return _NC_CACHE[key]


def _bin_bounds(rois: np.ndarray):
    f = np.float32
    rois = rois.astype(f)
    xs = np.round(rois[:, 1]) * f(SS)
    ys = np.round(rois[:, 2]) * f(SS)
    xe = np.round(rois[:, 3] + f(1.0)) * f(SS)
    ye = np.round(rois[:, 4] + f(1.0)) * f(SS)
    roi_w = np.maximum(xe - xs, f(0.1))
    roi_h = np.maximum(ye - ys, f(0.1))
    inv_gs = f(1.0) / f(GS)
    bin_w = (roi_w * inv_gs).astype(f)
    bin_h = (roi_h * inv_gs).astype(f)
    pidx = np.arange(GS, dtype=f)
    hstart = np.clip(np.floor(pidx[None, :] * bin_h[:, None] + ys[:, None]), 0, H)
    hend = np.clip(np.ceil((pidx[None, :] + f(1.0)) * bin_h[:, None] + ys[:, None]), 0, H)
    wstart = np.clip(np.floor(pidx[None, :] * bin_w[:, None] + xs[:, None]), 0, W)
    wend = np.clip(np.ceil((pidx[None, :] + f(1.0)) * bin_w[:, None] + xs[:, None]), 0, W)
    return hstart, hend, wstart, wend


def _shard(rois: np.ndarray):
    batch = rois[:, 0].astype(np.int32)
    order = np.argsort(batch, kind="stable")
    if R % N_CORES == 0:
        chunks = [order[i * (R // N_CORES) : (i + 1) * (R // N_CORES)] for i in range(N_CORES)]
        if all(len(np.unique(batch[c])) <= 2 for c in chunks):
            return chunks, (R // N_CORES + 127) // 128, batch
    chunks = [np.nonzero(batch == i)[0] for i in range(N_CORES)]
    maxc = max(len(c) for c in chunks)
    rt = (maxc + 127) // 128
    return chunks, rt, batch


def _sort_and_windows(rois, chunks):
    """Sort each core's ROIs by we6; compute global per-(t,q) windows.

    Returns (sorted chunks, Wp, X0) where X0[t][q] are compile-time
    window starts shared by all cores and Wp the common padded width."""
    hs, he, ws, we = _bin_bounds(rois)
    key = we[:, GS - 1]
    schunks = [c[np.argsort(key[c], kind="stable")] for c in chunks]
    x0 = np.full((2, GS), W, np.float64)
    x1 = np.zeros((2, GS), np.float64)
    for c in schunks:
        for t in range(2):
            ti = c[t * 128 : (t + 1) * 128]
            if len(ti) == 0:
                continue
            for q in range(GS):
                x0[t, q] = min(x0[t, q], ws[ti, q].min())
                x1[t, q] = max(x1[t, q], we[ti, q].max())
    wmax = int((x1 - x0).max())
    Wp = max(16, ((wmax + 7) // 8) * 8)
    full = (64, [[0] * GS for _ in range(2)])
    if Wp > 48:
        return schunks, *full
    X0 = [[int(min(max(x0[t, q], 0), W - Wp)) for q in range(GS)] for t in range(2)]
    for t in range(2):
        for q in range(GS):
            if x1[t, q] > X0[t][q] + Wp:
                return schunks, *full
    return schunks, Wp, X0


def _host_inputs(feat, rois, chunks, rt, batch, Wp, X0):
    hs, he, ws, we = _bin_bounds(rois)
    cnt_h = (he - hs).astype(np.float32)
    cnt_w = (we - ws).astype(np.float32)
    inv_h = np.where(cnt_h > 0, np.float32(1.0) / np.maximum(cnt_h, 1), 0).astype(np.float32)
    inv_w = np.where(cnt_w > 0, np.float32(1.0) / np.maximum(cnt_w, 1), 0).astype(np.float32)

    yi = np.arange(H, dtype=np.float32)
    xi = np.arange(W, dtype=np.float32)
    mask_h = ((yi[None, None, :] >= hs[:, :, None]) & (yi[None, None, :] < he[:, :, None])).astype(np.float32)
    mask_h *= inv_h[:, :, None]
    mask_w = ((xi[None, None, :] >= ws[:, :, None]) & (xi[None, None, :] < we[:, :, None])).astype(np.float32)
    mask_w *= inv_w[:, :, None]

    in_maps = []
    for core in range(N_CORES):
        idx = chunks[core]
        n_r = len(idx)
        imgs = np.unique(batch[idx])
        assert len(imgs) <= 2, f"core {core} spans {len(imgs)} images"
        iA = int(imgs[0])
        iB = int(imgs[1]) if len(imgs) > 1 else iA
        slot = (batch[idx] == iB).astype(np.int64) if iB != iA else np.zeros(n_r, np.int64)

        fpair = feat[[iA, iB]]  # [2, C, H, W] with C = (c, ph, pw)
        f6 = fpair.reshape(2, OD, GS, GS, H, W)
        # -> [(slot, y), (ph, c, q, x)]
        feat2 = np.ascontiguousarray(
            f6.transpose(0, 4, 2, 1, 3, 5).reshape(128, C * W)
        ).astype(NPBF16)

        rr = np.arange(n_r)
        rt_idx = rr // 128
        rp_idx = rr % 128

        mh_t = np.zeros((rt, 128, 2, GS, H), np.float32)
        mh_t[rt_idx, rp_idx, slot] = mask_h[idx]
        mh_host = np.ascontiguousarray(
            mh_t.transpose(2, 4, 0, 3, 1).reshape(128, rt * GS * 128)
        ).astype(NPBF16)

        # mwr: [r128, (t, q, c, x in window)]
        mw_t = np.zeros((rt, 128, GS, OD, Wp), np.float32)
        for t in range(rt):
            sel = rt_idx == t
            ridx = idx[sel]
            for q in range(GS):
                x0 = X0[t][q]
                mw_t[t, rp_idx[sel], q, :, :] = mask_w[ridx][:, None, q, x0 : x0 + Wp]
        mwr_host = np.ascontiguousarray(
            mw_t.transpose(1, 0, 2, 3, 4).reshape(128, rt * GS * OD * Wp)
        ).astype(NPBF16)
        mwr_host = np.concatenate([mwr_host, mwr_host], axis=1)  # j-duplicated

        in_maps.append({"feat2": feat2, "mh": mh_host, "mwr": mwr_host})
    return in_maps


def _run_cores(feat, rois, trace=False, reps=1, stages="full"):
    feat = np.ascontiguousarray(np.asarray(feat, dtype=np.float32))
    rois = np.asarray(rois, dtype=np.float32)
    assert feat.shape == (N_IMG, C, H, W), feat.shape
    assert rois.shape == (R, 5), rois.shape

    chunks, rt, batch = _shard(rois)
    assert rt == 2, rt
    chunks, Wp, X0 = _sort_and_windows(rois, chunks)
    cap = rt * 128
    nc = _get_nc(rt, reps, Wp, X0, stages)
    in_maps = _host_inputs(feat, rois, chunks, rt, batch, Wp, X0)

    res = run_bass_kernel_spmd(nc, in_maps, list(range(N_CORES)), trace=trace)

    out_full = np.zeros((R, OD, GS, GS), np.float32)
    for core in range(N_CORES):
        idx = chunks[core]
        o = np.asarray(res.results[core]["out"])
        # [128, (t, ph, q, c)] -> [cap, OD, ph, q]
        o = o.reshape(128, rt, GS, GS, OD).transpose(1, 0, 4, 2, 3).reshape(cap, OD, GS, GS)
        out_full[idx] = o[: len(idx)]
    return out_full, res


def kernel(feat: np.ndarray, rois: np.ndarray) -> np.ndarray:
    out, _ = _run_cores(feat, rois, trace=False)
    return out


# revision 3
# speedup vs baseline: 1.0048x; 1.0048x over previous
"""PSROIPool Trainium2 kernel v2: per-(tile,q) x-windows.

8-core SPMD data-parallel over ROIs (contiguous batch-sorted 256-ROI
chunks, <=2 images per core, 2-slot y-mask trick). Each core's 256 ROIs
are sorted by their last-bin right edge (we6) and split into 2 tiles of
128. For every (tile, q) a GLOBAL compile-time x-window [X0, X0+Wp) is
computed on the host (union over all cores, padded to a common width Wp)
that covers bin q of every ROI in that tile on every core. Stage 1 runs
one matmul per (tile, ph, q) with rhs = feat[ph, c=0..4, q, window]
(N = 5*Wp), so the masked x-reduction downstream only touches Wp
columns per bin instead of 56/64. mwr shrinks from [r, t*OD*GS*56] to
[r, t*GS*OD*Wp] (w-mask within the window, broadcast over c on host).
ScalarE casts PSUM->SBUF bf16; DVE does w-mask mul at 2x + bf16
add-tree + fp32 reduce. Falls back to full-width windows if the data
doesn't admit Wp <= 48.
"""

import os

import numpy as np
import ml_dtypes

import concourse.bass as bass
import concourse.bacc as bacc
import concourse.mybir as mybir
import concourse.tile as tile
from concourse.bass_utils import run_bass_kernel_spmd
from contextlib import ExitStack

N_IMG = 8
OD = 5
GS = 7
C = OD * GS * GS  # 245
H = W = 64
R = 2048
SS = 1.0 / 16.0
N_CORES = 8
F32 = mybir.dt.float32
BF16 = mybir.dt.bfloat16
NPBF16 = ml_dtypes.bfloat16

_NC_CACHE: dict = {}


def _build_nc(rt, reps, Wp, X0, stages="full"):
    """X0: [rt][GS] compile-time window starts; Wp: common width."""
    assert rt == 2
    nc = bacc.Bacc()
    chx = C * W  # 15680
    NW = OD * Wp          # matmul N per (t, ph, q)
    SLOT = 256 if NW <= 256 else 512  # psum cols per q slot
    assert NW <= SLOT
    PHSEG = GS * NW       # stg cols per (t, ph): (q, c, x)
    NSEG = rt * GS * OD   # segments per ph op: (t, q, c)

    feat2 = nc.declare_dram_parameter("feat2", [128, chx], BF16, isOutput=False)
    mh = nc.declare_dram_parameter("mh", [128, rt * GS * 128], BF16, isOutput=False)
    mwr = nc.declare_dram_parameter("mwr", [128, 2 * rt * GS * OD * Wp], BF16, isOutput=False)
    outp = nc.declare_dram_parameter("out", [128, rt * GS * GS * OD], F32, isOutput=True)

    with tile.TileContext(nc) as tc:
        with ExitStack() as ctx:
            pool = ctx.enter_context(tc.tile_pool(name="sb", bufs=1 if reps == 1 else 2))
            stp = ctx.enter_context(tc.tile_pool(name="stg", bufs=2))
            prp = ctx.enter_context(tc.tile_pool(name="prd", bufs=2))
            trp = ctx.enter_context(tc.tile_pool(name="tr", bufs=2))
            psp = ctx.enter_context(
                tc.tile_pool(
                    name="ps", bufs=2 if SLOT <= 256 else 1,
                    space=bass.MemorySpace.PSUM,
                )
            )

            for _rep in range(reps):
                mhT = pool.tile([128, rt * GS * 128], BF16, tag="mh")
                nc.sync.dma_start(mhT[:], mh[:])
                mwT = pool.tile([128, 2 * rt * GS * OD * Wp], BF16, tag="mw")
                nc.sync.dma_start(mwT[:], mwr[:])
                featT = pool.tile([128, chx], BF16, tag="feat")
                PHW = OD * GS * W  # 2240 cols per ph
                nch = int(os.environ.get("PSROI_FEAT_CHUNKS", "7"))
                bnd = [round(i * GS / nch) for i in range(nch + 1)]
                for i in range(nch):
                    a, b = bnd[i] * PHW, bnd[i + 1] * PHW
                    nc.sync.dma_start(featT[:, a:b], feat2[:, a:b])
                outT = pool.tile([128, rt * GS * GS * OD], F32, tag="out")

                featv = featT[:].rearrange(
                    "p (g c q x) -> p g c q x", g=GS, c=OD, q=GS
                )
                outv = outT[:].rearrange(
                    "p (t h q c) -> p t h q c", t=rt, h=GS, q=GS
                )

                for php in ((0, 1), (2, 3), (4, 5), (6,)):
                    nj = len(php)
                    stg = stp.tile([128, nj * rt * PHSEG], BF16, tag="stg")
                    for j, ph in enumerate(php):
                        for t in range(rt):
                            k = t * GS + ph
                            lhs = mhT[:, k * 128 : (k + 1) * 128]
                            ps = psp.tile([128, GS * SLOT], F32, tag="ps")
                            psv = ps[:].rearrange("p (q s) -> p q s", s=SLOT)
                            for q in range(GS):
                                x0 = X0[t][q]
                                rhs = featv[:, ph, :, q, x0 : x0 + Wp]
                                nc.tensor.matmul(
                                    psv[:, q, 0:NW], lhs, rhs, start=True, stop=True
                                )
                            if stages in ("act", "full"):
                                nc.scalar.copy(
                                    stg[
                                        :,
                                        (j * rt + t) * PHSEG : (j * rt + t + 1) * PHSEG,
                                    ],
                                    psv[:, :, 0:NW],
                                )
                            elif stages == "pe":
                                nc.vector.tensor_add(
                                    stg[:, t : t + 1], psv[:, 0, 0:1], psv[:, 1, 1:2]
                                )
                    if stages != "full":
                        continue
                    prod = prp.tile([128, nj * rt * PHSEG], BF16, tag="prd")
                    nc.vector.tensor_mul(prod[:], stg[:], mwT[:, 0 : nj * rt * PHSEG])
                    pv = prod[:].rearrange("p (s x) -> p s x", x=Wp)
                    w = Wp
                    src = pv
                    lvl = 0
                    while w % 2 == 0 and w > 5:
                        h = w // 2
                        tl = trp.tile([128, nj * NSEG * h], BF16, tag=f"t{lvl}")
                        tlv = tl[:].rearrange("p (s x) -> p s x", x=h)
                        nc.vector.tensor_add(tlv, src[:, :, 0:h], src[:, :, h:w])
                        src = tlv
                        w = h
                        lvl += 1
                    # tail: w in {3,4,5}; small add cascade beats the 1x reduce
                    outw = outv[:, :, php[0] : php[0] + nj, :, :].rearrange(
                        "p t j q c -> p j t q c"
                    )
                    ns = nj * NSEG

                    def _col(v, a, b):
                        return v[:, :, a:b].rearrange(
                            "p (j t q c) x -> p j t q (c x)", j=nj, t=rt, q=GS
                        )

                    if w == 5:
                        a1 = trp.tile([128, ns * 2], BF16, tag="ta")
                        a1v = a1[:].rearrange("p (s x) -> p s x", x=2)
                        nc.vector.tensor_add(a1v, src[:, :, 0:2], src[:, :, 2:4])
                        a2 = trp.tile([128, ns], BF16, tag="tb")
                        a2v = a2[:].rearrange("p (s x) -> p s x", x=1)
                        nc.vector.tensor_add(a2v, a1v[:, :, 0:1], a1v[:, :, 1:2])
                        nc.vector.tensor_add(outw, _col(a2v, 0, 1), _col(src, 4, 5))
                    elif w == 4:
                        a1 = trp.tile([128, ns * 2], BF16, tag="ta")
                        a1v = a1[:].rearrange("p (s x) -> p s x", x=2)
                        nc.vector.tensor_add(a1v, src[:, :, 0:2], src[:, :, 2:4])
                        nc.vector.tensor_add(outw, _col(a1v, 0, 1), _col(a1v, 1, 2))
                    else:  # w == 3
                        a1 = trp.tile([128, ns], BF16, tag="ta")
                        a1v = a1[:].rearrange("p (s x) -> p s x", x=1)
                        nc.vector.tensor_add(a1v, src[:, :, 0:1], src[:, :, 1:2])
                        nc.vector.tensor_add(outw, _col(a1v, 0, 1), _col(src, 2, 3))
                if stages == "full":
                    nc.sync.dma_start(outp[:], outT[:])

    nc.finalize()
    return nc


def _get_nc(rt, reps, Wp, X0, stages="full"):
    key = (rt, reps, Wp, tuple(map(tuple, X0)), stages)
    if key not in _NC_CACHE:
        _NC_CACHE[key] = _build_nc(rt, reps, Wp, X0, stages)
    return _NC_CACHE[key]


def _bin_bounds(rois: np.ndarray):
    f = np.float32
    rois = rois.astype(f)
    xs = np.round(rois[:, 1]) * f(SS)
    ys = np.round(rois[:, 2]) * f(SS)
    xe = np.round(rois[:, 3] + f(1.0)) * f(SS)
    ye = np.round(rois[:, 4] + f(1.0)) * f(SS)
    roi_w = np.maximum(xe - xs, f(0.1))
    roi_h = np.maximum(ye - ys, f(0.1))
    inv_gs = f(1.0) / f(GS)
    bin_w = (roi_w * inv_gs).astype(f)
    bin_h = (roi_h * inv_gs).astype(f)
    pidx = np.arange(GS, dtype=f)
    hstart = np.clip(np.floor(pidx[None, :] * bin_h[:, None] + ys[:, None]), 0, H)
    hend = np.clip(np.ceil((pidx[None, :] + f(1.0)) * bin_h[:, None] + ys[:, None]), 0, H)
    wstart = np.clip(np.floor(pidx[None, :] * bin_w[:, None] + xs[:, None]), 0, W)
    wend = np.clip(np.ceil((pidx[None, :] + f(1.0)) * bin_w[:, None] + xs[:, None]), 0, W)
    return hstart, hend, wstart, wend


def _shard(rois: np.ndarray):
    batch = rois[:, 0].astype(np.int32)
    order = np.argsort(batch, kind="stable")
    if R % N_CORES == 0:
        chunks = [order[i * (R // N_CORES) : (i + 1) * (R // N_CORES)] for i in range(N_CORES)]
        if all(len(np.unique(batch[c])) <= 2 for c in chunks):
            return chunks, (R // N_CORES + 127) // 128, batch
    chunks = [np.nonzero(batch == i)[0] for i in range(N_CORES)]
    maxc = max(len(c) for c in chunks)
    rt = (maxc + 127) // 128
    return chunks, rt, batch


def _sort_and_windows(rois, chunks):
    """Sort each core's ROIs by we6; compute global per-(t,q) windows.

    Returns (sorted chunks, Wp, X0) where X0[t][q] are compile-time
    window starts shared by all cores and Wp the common padded width."""
    hs, he, ws, we = _bin_bounds(rois)
    key = we[:, GS - 1]
    schunks = [c[np.argsort(key[c], kind="stable")] for c in chunks]
    x0 = np.full((2, GS), W, np.float64)
    x1 = np.zeros((2, GS), np.float64)
    for c in schunks:
        for t in range(2):
            ti = c[t * 128 : (t + 1) * 128]
            if len(ti) == 0:
                continue
            for q in range(GS):
                x0[t, q] = min(x0[t, q], ws[ti, q].min())
                x1[t, q] = max(x1[t, q], we[ti, q].max())
    wmax = int((x1 - x0).max())
    Wp = max(16, ((wmax + 7) // 8) * 8)
    full = (64, [[0] * GS for _ in range(2)])
    if Wp > 48:
        return schunks, *full
    X0 = [[int(min(max(x0[t, q], 0), W - Wp)) for q in range(GS)] for t in range(2)]
    for t in range(2):
        for q in range(GS):
            if x1[t, q] > X0[t][q] + Wp:
                return schunks, *full
    return schunks, Wp, X0


def _host_inputs(feat, rois, chunks, rt, batch, Wp, X0):
    hs, he, ws, we = _bin_bounds(rois)
    cnt_h = (he - hs).astype(np.float32)
    cnt_w = (we - ws).astype(np.float32)
    inv_h = np.where(cnt_h > 0, np.float32(1.0) / np.maximum(cnt_h, 1), 0).astype(np.float32)
    inv_w = np.where(cnt_w > 0, np.float32(1.0) / np.maximum(cnt_w, 1), 0).astype(np.float32)

    yi = np.arange(H, dtype=np.float32)
    xi = np.arange(W, dtype=np.float32)
    mask_h = ((yi[None, None, :] >= hs[:, :, None]) & (yi[None, None, :] < he[:, :, None])).astype(np.float32)
    mask_h *= inv_h[:, :, None]
    mask_w = ((xi[None, None, :] >= ws[:, :, None]) & (xi[None, None, :] < we[:, :, None])).astype(np.float32)
    mask_w *= inv_w[:, :, None]

    in_maps = []
    for core in range(N_CORES):
        idx = chunks[core]
        n_r = len(idx)
        imgs = np.unique(batch[idx])
        assert len(imgs) <= 2, f"core {core} spans {len(imgs)} images"
        iA = int(imgs[0])
        iB = int(imgs[1]) if len(imgs) > 1 else iA
        slot = (batch[idx] == iB).astype(np.int64) if iB != iA else np.zeros(n_r, np.int64)

        fpair = feat[[iA, iB]]  # [2, C, H, W] with C = (c, ph, pw)
        f6 = fpair.reshape(2, OD, GS, GS, H, W)
        # -> [(slot, y), (ph, c, q, x)]
        feat2 = np.ascontiguousarray(
            f6.transpose(0, 4, 2, 1, 3, 5).reshape(128, C * W)
        ).astype(NPBF16)

        rr = np.arange(n_r)
        rt_idx = rr // 128
        rp_idx = rr % 128

        mh_t = np.zeros((rt, 128, 2, GS, H), np.float32)
        mh_t[rt_idx, rp_idx, slot] = mask_h[idx]
        mh_host = np.ascontiguousarray(
            mh_t.transpose(2, 4, 0, 3, 1).reshape(128, rt * GS * 128)
        ).astype(NPBF16)

        # mwr: [r128, (t, q, c, x in window)]
        mw_t = np.zeros((rt, 128, GS, OD, Wp), np.float32)
        for t in range(rt):
            sel = rt_idx == t
            ridx = idx[sel]
            for q in range(GS):
                x0 = X0[t][q]
                mw_t[t, rp_idx[sel], q, :, :] = mask_w[ridx][:, None, q, x0 : x0 + Wp]
        mwr_host = np.ascontiguousarray(
            mw_t.transpose(1, 0, 2, 3, 4).reshape(128, rt * GS * OD * Wp)
        ).astype(NPBF16)
        mwr_host = np.concatenate([mwr_host, mwr_host], axis=1)  # j-duplicated

        in_maps.append({"feat2": feat2, "mh": mh_host, "mwr": mwr_host})
    return in_maps


def _run_cores(feat, rois, trace=False, reps=1, stages="full"):
    feat = np.ascontiguousarray(np.asarray(feat, dtype=np.float32))
    rois = np.asarray(rois, dtype=np.float32)
    assert feat.shape == (N_IMG, C, H, W), feat.shape
    assert rois.shape == (R, 5), rois.shape

    chunks, rt, batch = _shard(rois)
    assert rt == 2, rt
    chunks, Wp, X0 = _sort_and_windows(rois, chunks)
    cap = rt * 128
    nc = _get_nc(rt, reps, Wp, X0, stages)
    in_maps = _host_inputs(feat, rois, chunks, rt, batch, Wp, X0)

    res = run_bass_kernel_spmd(nc, in_maps, list(range(N_CORES)), trace=trace)

    out_full = np.zeros((R, OD, GS, GS), np.float32)
    for core in range(N_CORES):
        idx = chunks[core]
        o = np.asarray(res.results[core]["out"])
        # [128, (t, ph, q, c)] -> [cap, OD, ph, q]
        o = o.reshape(128, rt, GS, GS, OD).transpose(1, 0, 4, 2, 3).reshape(cap, OD, GS, GS)
        out_full[idx] = o[: len(idx)]
    return out_full, res


def kernel(feat: np.ndarray, rois: np.ndarray) -> np.ndarray:
    out, _ = _run_cores(feat, rois, trace=False)
    return out


# revision 4
# speedup vs baseline: 1.1524x; 1.1469x over previous
"""PSROIPool Trainium2 kernel v2: per-(tile,q) x-windows.

8-core SPMD data-parallel over ROIs (contiguous batch-sorted 256-ROI
chunks, <=2 images per core, 2-slot y-mask trick). Each core's 256 ROIs
are sorted by their last-bin right edge (we6) and split into 2 tiles of
128. For every (tile, q) a GLOBAL compile-time x-window [X0, X0+Wp) is
computed on the host (union over all cores, padded to a common width Wp)
that covers bin q of every ROI in that tile on every core. Stage 1 runs
one matmul per (tile, ph, q) with rhs = feat[ph, c=0..4, q, window]
(N = 5*Wp), so the masked x-reduction downstream only touches Wp
columns per bin instead of 56/64. mwr shrinks from [r, t*OD*GS*56] to
[r, t*GS*OD*Wp] (w-mask within the window, broadcast over c on host).
ScalarE casts PSUM->SBUF bf16; DVE does w-mask mul at 2x + bf16
add-tree + fp32 reduce. Falls back to full-width windows if the data
doesn't admit Wp <= 48.
"""

import os

import numpy as np
import ml_dtypes

import concourse.bass as bass
import concourse.bacc as bacc
import concourse.mybir as mybir
import concourse.tile as tile
from concourse.bass_utils import run_bass_kernel_spmd
from contextlib import ExitStack

N_IMG = 8
OD = 5
GS = 7
C = OD * GS * GS  # 245
H = W = 64
R = 2048
SS = 1.0 / 16.0
N_CORES = 8
F32 = mybir.dt.float32
BF16 = mybir.dt.bfloat16
NPBF16 = ml_dtypes.bfloat16

_NC_CACHE: dict = {}


def _build_nc(rt, reps, Wp, X0, stages="full"):
    """X0: [rt][GS] compile-time window starts; Wp: common width."""
    assert rt == 2
    nc = bacc.Bacc()
    chx = C * W  # 15680
    NW = OD * Wp          # matmul N per (t, ph, q)
    SLOT = 256 if NW <= 256 else 512  # psum cols per q slot
    assert NW <= SLOT
    PHSEG = GS * NW       # stg cols per (t, ph): (q, c, x)
    NSEG = rt * GS * OD   # segments per ph op: (t, q, c)

    feat2 = nc.declare_dram_parameter("feat2", [128, chx], BF16, isOutput=False)
    mh = nc.declare_dram_parameter("mh", [128, rt * GS * 128], BF16, isOutput=False)
    mwr = nc.declare_dram_parameter("mwr", [128, 2 * rt * GS * OD * Wp], BF16, isOutput=False)
    outp = nc.declare_dram_parameter("out", [128, rt * GS * GS * OD], F32, isOutput=True)

    with tile.TileContext(nc) as tc:
        with ExitStack() as ctx:
            pool = ctx.enter_context(tc.tile_pool(name="sb", bufs=1 if reps == 1 else 2))
            stp = ctx.enter_context(tc.tile_pool(name="stg", bufs=2))
            prp = ctx.enter_context(tc.tile_pool(name="prd", bufs=2))
            # tree tiles are DVE-produced and DVE-consumed (serial engine):
            # single-buffered is free and halves SBUF pressure
            trp = ctx.enter_context(tc.tile_pool(name="tr", bufs=1))
            psp = ctx.enter_context(
                tc.tile_pool(
                    name="ps", bufs=2 if SLOT <= 256 else 1,
                    space=bass.MemorySpace.PSUM,
                )
            )

            for _rep in range(reps):
                mhT = pool.tile([128, rt * GS * 128], BF16, tag="mh")
                nc.sync.dma_start(mhT[:], mh[:])
                mwT = pool.tile([128, 2 * rt * GS * OD * Wp], BF16, tag="mw")
                nc.sync.dma_start(mwT[:], mwr[:])
                featT = pool.tile([128, chx], BF16, tag="feat")
                PHW = OD * GS * W  # 2240 cols per ph
                nch = int(os.environ.get("PSROI_FEAT_CHUNKS", "7"))
                bnd = [round(i * GS / nch) for i in range(nch + 1)]
                for i in range(nch):
                    a, b = bnd[i] * PHW, bnd[i + 1] * PHW
                    nc.sync.dma_start(featT[:, a:b], feat2[:, a:b])
                outT = pool.tile([128, rt * GS * GS * OD], F32, tag="out")

                featv = featT[:].rearrange(
                    "p (g c q x) -> p g c q x", g=GS, c=OD, q=GS
                )
                outv = outT[:].rearrange(
                    "p (t h q c) -> p t h q c", t=rt, h=GS, q=GS
                )

                # per-ph-group: matmuls, PSUM->SBUF copy, mask-mul, first
                # tree level into a shared t1all buffer; the lower tree
                # levels then run once over all groups (fewer DVE ops).
                NST = GS * rt * OD * GS  # 490 segments total (ph, t, q, c)
                wh1 = Wp // 2
                t1all = trp.tile([128, NST * wh1], BF16, tag="t1all")
                groups = ((0, 1), (2, 3), (4, 5), (6,))
                for php in groups:
                    nj = len(php)
                    stg = stp.tile([128, nj * rt * PHSEG], BF16, tag="stg")
                    for j, ph in enumerate(php):
                        for t in range(rt):
                            k = t * GS + ph
                            lhs = mhT[:, k * 128 : (k + 1) * 128]
                            ps = psp.tile([128, GS * SLOT], F32, tag="ps")
                            psv = ps[:].rearrange("p (q s) -> p q s", s=SLOT)
                            for q in range(GS):
                                x0 = X0[t][q]
                                rhs = featv[:, ph, :, q, x0 : x0 + Wp]
                                nc.tensor.matmul(
                                    psv[:, q, 0:NW], lhs, rhs, start=True, stop=True
                                )
                            if stages in ("act", "full"):
                                nc.scalar.copy(
                                    stg[
                                        :,
                                        (j * rt + t) * PHSEG : (j * rt + t + 1) * PHSEG,
                                    ],
                                    psv[:, :, 0:NW],
                                )
                            elif stages == "pe":
                                nc.vector.tensor_add(
                                    stg[:, t : t + 1], psv[:, 0, 0:1], psv[:, 1, 1:2]
                                )
                    if stages != "full":
                        continue
                    prod = prp.tile([128, nj * rt * PHSEG], BF16, tag="prd")
                    nc.vector.tensor_mul(prod[:], stg[:], mwT[:, 0 : nj * rt * PHSEG])
                    pv = prod[:].rearrange("p (s x) -> p s x", x=Wp)
                    off = php[0] * rt * OD * GS * wh1
                    ncols = nj * rt * OD * GS * wh1
                    t1v = t1all[:, off : off + ncols].rearrange(
                        "p (s x) -> p s x", x=wh1
                    )
                    nc.vector.tensor_add(t1v, pv[:, :, 0:wh1], pv[:, :, wh1:Wp])
                if stages == "full":
                    # lower levels over all 490 segments at once; odd level
                    # widths are padded to a multiple of 4 so segment strides
                    # stay 4B-aligned (keeps the DVE 2x mode)
                    w = wh1
                    src = t1all[:].rearrange("p (s x) -> p s x", x=wh1)
                    lvl = 0
                    while w % 2 == 0 and w > 5:
                        h = w // 2
                        pad = h if h % 2 == 0 else ((h + 3) // 4) * 4
                        tl = trp.tile([128, NST * pad], BF16, tag=f"t{lvl}")
                        tlv = tl[:].rearrange("p (s x) -> p s x", x=pad)
                        nc.vector.tensor_add(
                            tlv[:, :, 0:h], src[:, :, 0:h], src[:, :, h:w]
                        )
                        src = tlv
                        w = h
                        lvl += 1
                    # tail: w in {3,4,5,9}; out is (ph, t, q, c)-permuted outT
                    outw = outv.rearrange("p t h q c -> p h t q c")

                    def _col(v, a, b):
                        return v[:, :, a:b].rearrange(
                            "p (h t q c) x -> p h t q (c x)", h=GS, t=rt, q=GS
                        )

                    rem = None  # odd leftover column, added in the last op
                    if w == 9:
                        rem = (src, 8, 9)
                        b1 = trp.tile([128, NST * 4], BF16, tag="tc")
                        b1v = b1[:].rearrange("p (s x) -> p s x", x=4)
                        nc.vector.tensor_add(b1v, src[:, :, 0:4], src[:, :, 4:8])
                        src = b1v
                        w = 4
                    if w == 5:
                        rem = (src, 4, 5)
                        w = 4
                    if w == 4:
                        a1 = trp.tile([128, NST * 2], BF16, tag="ta")
                        a1v = a1[:].rearrange("p (s x) -> p s x", x=2)
                        nc.vector.tensor_add(a1v, src[:, :, 0:2], src[:, :, 2:4])
                        if rem is None:
                            nc.vector.tensor_add(
                                outw, _col(a1v, 0, 1), _col(a1v, 1, 2)
                            )
                        else:
                            a2 = trp.tile([128, NST], BF16, tag="tb")
                            a2v = a2[:].rearrange("p (s x) -> p s x", x=1)
                            nc.vector.tensor_add(
                                a2v, a1v[:, :, 0:1], a1v[:, :, 1:2]
                            )
                            nc.vector.tensor_add(
                                outw, _col(a2v, 0, 1), _col(rem[0], rem[1], rem[2])
                            )
                    else:  # w == 3
                        a1 = trp.tile([128, NST], BF16, tag="ta")
                        a1v = a1[:].rearrange("p (s x) -> p s x", x=1)
                        nc.vector.tensor_add(a1v, src[:, :, 0:1], src[:, :, 1:2])
                        nc.vector.tensor_add(outw, _col(a1v, 0, 1), _col(src, 2, 3))
                if stages == "full":
                    nc.sync.dma_start(outp[:], outT[:])

    nc.finalize()
    return nc


def _get_nc(rt, reps, Wp, X0, stages="full"):
    key = (rt, reps, Wp, tuple(map(tuple, X0)), stages)
    if key not in _NC_CACHE:
        _NC_CACHE[key] = _build_nc(rt, reps, Wp, X0, stages)
    return _NC_CACHE[key]


def _bin_bounds(rois: np.ndarray):
    f = np.float32
    rois = rois.astype(f)
    xs = np.round(rois[:, 1]) * f(SS)
    ys = np.round(rois[:, 2]) * f(SS)
    xe = np.round(rois[:, 3] + f(1.0)) * f(SS)
    ye = np.round(rois[:, 4] + f(1.0)) * f(SS)
    roi_w = np.maximum(xe - xs, f(0.1))
    roi_h = np.maximum(ye - ys, f(0.1))
    inv_gs = f(1.0) / f(GS)
    bin_w = (roi_w * inv_gs).astype(f)
    bin_h = (roi_h * inv_gs).astype(f)
    pidx = np.arange(GS, dtype=f)
    hstart = np.clip(np.floor(pidx[None, :] * bin_h[:, None] + ys[:, None]), 0, H)
    hend = np.clip(np.ceil((pidx[None, :] + f(1.0)) * bin_h[:, None] + ys[:, None]), 0, H)
    wstart = np.clip(np.floor(pidx[None, :] * bin_w[:, None] + xs[:, None]), 0, W)
    wend = np.clip(np.ceil((pidx[None, :] + f(1.0)) * bin_w[:, None] + xs[:, None]), 0, W)
    return hstart, hend, wstart, wend


def _shard(rois: np.ndarray):
    batch = rois[:, 0].astype(np.int32)
    order = np.argsort(batch, kind="stable")
    if R % N_CORES == 0:
        chunks = [order[i * (R // N_CORES) : (i + 1) * (R // N_CORES)] for i in range(N_CORES)]
        if all(len(np.unique(batch[c])) <= 2 for c in chunks):
            return chunks, (R // N_CORES + 127) // 128, batch
    chunks = [np.nonzero(batch == i)[0] for i in range(N_CORES)]
    maxc = max(len(c) for c in chunks)
    rt = (maxc + 127) // 128
    return chunks, rt, batch


def _sort_and_windows(rois, chunks):
    """Sort each core's ROIs by we6; compute global per-(t,q) windows.

    Returns (sorted chunks, Wp, X0) where X0[t][q] are compile-time
    window starts shared by all cores and Wp the common padded width."""
    hs, he, ws, we = _bin_bounds(rois)
    key = np.float32(0.8) * we[:, GS - 1] + np.float32(0.2) * ws[:, 0]
    schunks = [c[np.argsort(key[c], kind="stable")] for c in chunks]
    x0 = np.full((2, GS), W, np.float64)
    x1 = np.zeros((2, GS), np.float64)
    for c in schunks:
        for t in range(2):
            ti = c[t * 128 : (t + 1) * 128]
            if len(ti) == 0:
                continue
            for q in range(GS):
                x0[t, q] = min(x0[t, q], ws[ti, q].min())
                x1[t, q] = max(x1[t, q], we[ti, q].max())
    wmax = int((x1 - x0).max())
    # admissible widths: the add-tree tail must land on {3,4,5,9}
    Wp = next((w for w in (16, 20, 24, 32, 36, 40, 48) if w >= wmax), 64)
    full = (64, [[0] * GS for _ in range(2)])
    if Wp > 48:
        return schunks, *full
    X0 = [[int(min(max(x0[t, q], 0), W - Wp)) for q in range(GS)] for t in range(2)]
    for t in range(2):
        for q in range(GS):
            if x1[t, q] > X0[t][q] + Wp:
                return schunks, *full
    return schunks, Wp, X0


def _host_inputs(feat, rois, chunks, rt, batch, Wp, X0):
    hs, he, ws, we = _bin_bounds(rois)
    cnt_h = (he - hs).astype(np.float32)
    cnt_w = (we - ws).astype(np.float32)
    inv_h = np.where(cnt_h > 0, np.float32(1.0) / np.maximum(cnt_h, 1), 0).astype(np.float32)
    inv_w = np.where(cnt_w > 0, np.float32(1.0) / np.maximum(cnt_w, 1), 0).astype(np.float32)

    yi = np.arange(H, dtype=np.float32)
    xi = np.arange(W, dtype=np.float32)
    mask_h = ((yi[None, None, :] >= hs[:, :, None]) & (yi[None, None, :] < he[:, :, None])).astype(np.float32)
    mask_h *= inv_h[:, :, None]
    mask_w = ((xi[None, None, :] >= ws[:, :, None]) & (xi[None, None, :] < we[:, :, None])).astype(np.float32)
    mask_w *= inv_w[:, :, None]

    in_maps = []
    for core in range(N_CORES):
        idx = chunks[core]
        n_r = len(idx)
        imgs = np.unique(batch[idx])
        assert len(imgs) <= 2, f"core {core} spans {len(imgs)} images"
        iA = int(imgs[0])
        iB = int(imgs[1]) if len(imgs) > 1 else iA
        slot = (batch[idx] == iB).astype(np.int64) if iB != iA else np.zeros(n_r, np.int64)

        fpair = feat[[iA, iB]]  # [2, C, H, W] with C = (c, ph, pw)
        f6 = fpair.reshape(2, OD, GS, GS, H, W)
        # -> [(slot, y), (ph, c, q, x)]
        feat2 = np.ascontiguousarray(
            f6.transpose(0, 4, 2, 1, 3, 5).reshape(128, C * W)
        ).astype(NPBF16)

        rr = np.arange(n_r)
        rt_idx = rr // 128
        rp_idx = rr % 128

        mh_t = np.zeros((rt, 128, 2, GS, H), np.float32)
        mh_t[rt_idx, rp_idx, slot] = mask_h[idx]
        mh_host = np.ascontiguousarray(
            mh_t.transpose(2, 4, 0, 3, 1).reshape(128, rt * GS * 128)
        ).astype(NPBF16)

        # mwr: [r128, (t, q, c, x in window)]
        mw_t = np.zeros((rt, 128, GS, OD, Wp), np.float32)
        for t in range(rt):
            sel = rt_idx == t
            ridx = idx[sel]
            for q in range(GS):
                x0 = X0[t][q]
                mw_t[t, rp_idx[sel], q, :, :] = mask_w[ridx][:, None, q, x0 : x0 + Wp]
        mwr_host = np.ascontiguousarray(
            mw_t.transpose(1, 0, 2, 3, 4).reshape(128, rt * GS * OD * Wp)
        ).astype(NPBF16)
        mwr_host = np.concatenate([mwr_host, mwr_host], axis=1)  # j-duplicated

        in_maps.append({"feat2": feat2, "mh": mh_host, "mwr": mwr_host})
    return in_maps


def _run_cores(feat, rois, trace=False, reps=1, stages="full"):
    feat = np.ascontiguousarray(np.asarray(feat, dtype=np.float32))
    rois = np.asarray(rois, dtype=np.float32)
    assert feat.shape == (N_IMG, C, H, W), feat.shape
    assert rois.shape == (R, 5), rois.shape

    chunks, rt, batch = _shard(rois)
    assert rt == 2, rt
    chunks, Wp, X0 = _sort_and_windows(rois, chunks)
    cap = rt * 128
    nc = _get_nc(rt, reps, Wp, X0, stages)
    in_maps = _host_inputs(feat, rois, chunks, rt, batch, Wp, X0)

    res = run_bass_kernel_spmd(nc, in_maps, list(range(N_CORES)), trace=trace)

    out_full = np.zeros((R, OD, GS, GS), np.float32)
    for core in range(N_CORES):
        idx = chunks[core]
        o = np.asarray(res.results[core]["out"])
        # [128, (t, ph, q, c)] -> [cap, OD, ph, q]
        o = o.reshape(128, rt, GS, GS, OD).transpose(1, 0, 4, 2, 3).reshape(cap, OD, GS, GS)
        out_full[idx] = o[: len(idx)]
    return out_full, res


def kernel(feat: np.ndarray, rois: np.ndarray) -> np.ndarray:
    out, _ = _run_cores(feat, rois, trace=False)
    return out
